# revision 1
# baseline (speedup 1.0000x reference)
"""EnhancedGovernanceAttention Trainium2 kernel (8 NeuronCores, SPMD).

Sharding: core c owns heads {2c, 2c+1} for BOTH batches (policy_mask is
per-head and batch-shared, so each policy slice is loaded once per core).
Each core computes its heads' attention and a row-parallel partial of the
Wo projection; the host sums the 8 partials and adds bo.

Math notes (vs the jax reference):
 - softmax max-subtraction is dropped: scores ~ N(0,1) + bias in [0,0.2],
   so exp() cannot overflow in fp32; softmax is shift-invariant.
 - log1p memory bias: softmax(s + log(w)) == (w * exp(s)) / sum(w * exp(s))
   with w = 1 + GS*mw + 1e-8, so w is folded into V rows and into the
   denominator matmul -- no per-score log bias needed.
 - causal mask: only lower-triangle k-tiles are computed; the intra-tile
   diagonal mask is baked into the (bf16) policy bias as -40.
 - scores are computed TRANSPOSED ([k, q]) so the PV matmul directly
   yields attn^T, which is the lhsT the output projection needs.
 - x^T is produced by bf16 hi/lo DMA-transposes + one DVE add (exact to
   ~2^-16 relative), avoiding PE-transpose traffic for x.
 - matmuls run in float32r (~1.8e-4 quantization, 4x faster than fp32).
"""

import numpy as np
import ml_dtypes
from contextlib import ExitStack

import concourse.bass as bass
import concourse.tile as tile
from concourse import bacc, mybir
from concourse.bass_utils import run_bass_kernel_spmd
from concourse.masks import make_identity

B, S, D, H, HD = 2, 2048, 2048, 16, 128
GS = 0.1
ROPE_BASE = 10000.0
NCORES = 8
HPC = H // NCORES          # heads per core = 2
SCALE = float(HD) ** -0.5
DT = D // 128              # 16 d-tiles
ST = S // 128              # 16 s-tiles (also k-tiles)
QB = 512                   # q-block width (phase B)
NQB = S // QB              # 4 q-blocks
SB = 256                   # s-block width (phase A panels)
NSB = S // SB              # 8 s-blocks
MASK_NEG = -40.0
SLAB_K = 2                 # k-tiles per bias slab load

F32 = mybir.dt.float32
F32R = mybir.dt.float32r
BF16 = mybir.dt.bfloat16

_CACHE = {}


def build_nc():
    nc = bacc.Bacc("TRN2", target_bir_lowering=False, debug=False,
                   num_devices=NCORES)

    d_xhi = nc.dram_tensor("xhi", [B, S, D], BF16, kind="ExternalInput").ap()
    d_xlo = nc.dram_tensor("xlo", [B, S, D], BF16, kind="ExternalInput").ap()
    d_wq = nc.dram_tensor("wq", [D, HPC * HD], F32R, kind="ExternalInput").ap()
    d_wk = nc.dram_tensor("wk", [D, HPC * HD], F32R, kind="ExternalInput").ap()
    d_wv = nc.dram_tensor("wv", [D, HPC * HD], F32R, kind="ExternalInput").ap()
    d_wo = nc.dram_tensor("wo", [HPC * HD, D], F32R, kind="ExternalInput").ap()
    d_bias = nc.dram_tensor("biasT", [HPC, S, S], BF16, kind="ExternalInput").ap()
    d_wr = nc.dram_tensor("wr", [B, S], F32R, kind="ExternalInput").ap()
    d_w32 = nc.dram_tensor("w32", [B, S], F32, kind="ExternalInput").ap()
    d_cs = nc.dram_tensor("cs", [128, S], F32, kind="ExternalInput").ap()
    d_y = nc.dram_tensor("y", [B, S, D], F32, kind="ExternalOutput").ap()

    with tile.TileContext(nc) as tc, ExitStack() as ctx:
        consts = ctx.enter_context(tc.tile_pool(name="consts", bufs=1))
        wpool = ctx.enter_context(tc.tile_pool(name="wpool", bufs=1))
        qkv = ctx.enter_context(tc.tile_pool(name="qkv", bufs=1))
        panels = ctx.enter_context(tc.tile_pool(name="panels", bufs=2))
        hilo = ctx.enter_context(tc.tile_pool(name="hilo", bufs=2))
        hilo1 = ctx.enter_context(tc.tile_pool(name="hilo1", bufs=1))
        rope = ctx.enter_context(tc.tile_pool(name="rope", bufs=1))
        slabs = ctx.enter_context(tc.tile_pool(name="slabs", bufs=3))
        expp = ctx.enter_context(tc.tile_pool(name="expp", bufs=4))
        normp = ctx.enter_context(tc.tile_pool(name="normp", bufs=1))
        outp = ctx.enter_context(tc.tile_pool(name="outp", bufs=4))
        psum = ctx.enter_context(tc.tile_pool(name="psum", bufs=3, space="PSUM"))
        psum_pv = ctx.enter_context(tc.tile_pool(name="psum_pv", bufs=3, space="PSUM"))
        psum_l = ctx.enter_context(tc.tile_pool(name="psum_l", bufs=2, space="PSUM"))

        def emit_panel(b, sb_i):
            blk = slice(sb_i * SB, sb_i * SB + SB)
            panel = panels.tile([128, DT, SB], F32R, tag="panel", name="panel")
            thi = hilo.tile([128, DT, SB], BF16, tag="thi", name="thi")
            tlo = hilo1.tile([128, DT, SB], BF16, tag="tlo", name="tlo")
            nc.sync.dma_start_transpose(thi, d_xhi[b, blk, :])
            nc.sync.dma_start_transpose(tlo, d_xlo[b, blk, :])
            half = DT // 2
            nc.vector.tensor_add(
                panel[:, :half, :], thi[:, :half, :], tlo[:, :half, :])
            nc.gpsimd.tensor_add(
                panel[:, half:, :], thi[:, half:, :], tlo[:, half:, :])
            return panel

        panel_cache = {}

        # ---------------- constants (emission order = priority) ----------------
        t_w = {}
        for name, dram in (("wq", d_wq), ("wk", d_wk), ("wv", d_wv)):
            t = wpool.tile([128, DT, HPC * HD], F32R, tag=name, name=name)
            nc.gpsimd.dma_start(t, dram.rearrange("(t p) c -> p t c", p=128))
            t_w[name] = t
        t_cs = consts.tile([128, S], F32, tag="cs")
        nc.gpsimd.dma_start(t_cs, d_cs)
        ident = consts.tile([128, 128], F32, tag="ident")
        make_identity(nc, ident)
        ident_bf = consts.tile([128, 128], BF16, tag="ident_bf")
        make_identity(nc, ident_bf)
        t_w32 = consts.tile([128, B, ST], F32, tag="w32")
        nc.gpsimd.dma_start(t_w32, d_w32.rearrange("b (t p) -> p b t", p=128))
        t_wr = consts.tile([128, B, ST], F32R, tag="wr")
        nc.gpsimd.dma_start(t_wr, d_wr.rearrange("b (t p) -> p b t", p=128))
        t_wo = consts.tile([128, HPC, D], F32R, tag="wo")
        nc.gpsimd.dma_start(t_wo, d_wo.rearrange("(h p) c -> p h c", p=128))

        def emit_c_unit(attnT_ref, b_ref, st, nb):
            ss = slice(st * 128, (st + 1) * 128)
            ns = slice(nb * 512, (nb + 1) * 512)
            ops = psum_pv.tile([128, 512], F32, tag="pv", name="ops")
            for h in range(HPC):
                nc.tensor.matmul(
                    ops, attnT_ref[h][:, ss], t_wo[:, h, ns],
                    start=(h == 0), stop=(h == HPC - 1))
            ob = outp.tile([128, 512], F32, tag="ob")
            nc.vector.tensor_copy(ob, ops)
            nc.scalar.dma_start(d_y[b_ref, ss, ns], ob)

        pending_c = []
        for b in range(B):
            # ============ phase A: x^T panels -> q^T,k^T (RoPE), v ============
            qT = {}
            kT = {}
            vv = {}
            for h in range(HPC):
                qT[h] = qkv.tile([128, S], F32R, tag=f"qT{h}", name=f"qT{h}")
                kT[h] = qkv.tile([128, S], F32R, tag=f"kT{h}", name=f"kT{h}")
                vv[h] = qkv.tile([128, ST, HD], F32R, tag=f"v{h}", name=f"v{h}")

            for sb_i in range(NSB):
                s0 = sb_i * SB
                blk = slice(s0, s0 + SB)
                if (b, sb_i) in panel_cache:
                    panel = panel_cache.pop((b, sb_i))
                else:
                    panel = emit_panel(b, sb_i)

                for h in range(HPC):
                    hc = slice(h * HD, (h + 1) * HD)
                    # --- q^T and k^T with fused RoPE ---
                    for name, dest in (("wq", qT[h]), ("wk", kT[h])):
                        ps = psum.tile([128, SB], F32, tag="mm")
                        for dt in range(DT):
                            nc.tensor.matmul(
                                ps, t_w[name][:, dt, hc], panel[:, dt, :],
                                start=(dt == 0), stop=(dt == DT - 1))
                        t1 = rope.tile([128, SB], F32, tag="t1")
                        t2 = rope.tile([128, SB], F32, tag="t2")
                        # cs rows 0-63 = sinT, rows 64-127 = cosT
                        nc.vector.tensor_mul(
                            t1[0:64, :], ps[0:64, :], t_cs[64:128, blk])
                        nc.vector.tensor_mul(
                            t1[64:128, :], ps[64:128, :], t_cs[64:128, blk])
                        nc.vector.tensor_mul(
                            t2[0:64, :], ps[64:128, :], t_cs[0:64, blk])
                        nc.vector.tensor_mul(
                            t2[64:128, :], ps[0:64, :], t_cs[0:64, blk])
                        # dest = [x1*c - x2*s ; x2*c + x1*s]
                        nc.gpsimd.tensor_sub(
                            dest[0:64, blk], t1[0:64, :], t2[0:64, :])
                        nc.gpsimd.tensor_add(
                            dest[64:128, blk], t1[64:128, :], t2[64:128, :])
                    # --- v (natural layout) via PE transpose of v^T ---
                    ps = psum.tile([128, SB], F32, tag="mm")
                    for dt in range(DT):
                        nc.tensor.matmul(
                            ps, t_w["wv"][:, dt, hc], panel[:, dt, :],
                            start=(dt == 0), stop=(dt == DT - 1))
                    svt = normp.tile([128, SB], F32, tag="svt")
                    nc.scalar.copy(svt, ps)
                    vch = psum.tile([128, SB // 128, 128], F32, tag="mm")
                    for c4 in range(SB // 128):
                        nc.tensor.transpose(
                            vch[:, c4, :], svt[:, c4 * 128:(c4 + 1) * 128], ident)
                    for c4 in range(SB // 128):
                        stile = (s0 // 128) + c4
                        nc.scalar.activation(
                            vv[h][:, stile, :], vch[:, c4, :],
                            mybir.ActivationFunctionType.Copy,
                            scale=t_w32[:, b, stile:stile + 1])
                    # drain carried output units from the previous batch
                    if pending_c:
                        emit_c_unit(*pending_c.pop(0))

            # ====== phases B+C software-pipelined over q-blocks ======
            attnT = qT  # norm(j,h) overwrites qT[h][:, qs] after its last read
            for j in range(NQB):
                qs = slice(j * QB, (j + 1) * QB)
                nk = 4 * (j + 1)          # causal: k-tiles 0..nk-1
                steps_left = HPC * nk
                for h in range(HPC):
                    pv = psum_pv.tile([128, QB], F32, tag="pv")
                    lps = psum_l.tile([1, QB], F32, tag="l", name="lps")
                    for g in range((nk + SLAB_K - 1) // SLAB_K):
                        n = min(SLAB_K, nk - g * SLAB_K)
                        slab = slabs.tile([128, SLAB_K, QB], BF16, tag="slab")
                        k0 = g * SLAB_K * 128
                        slab_eng = nc.gpsimd if g % 2 == 0 else nc.scalar
                        slab_eng.dma_start(
                            slab[:, :n, :],
                            d_bias[h, k0:k0 + n * 128, qs].rearrange(
                                "(m p) q -> p m q", p=128))
                        for ml in range(n):
                            m = g * SLAB_K + ml
                            # columns q < 128*m are fully causal-masked; skip
                            # them, but keep N >= 256 (f32r speed) when useful
                            off = max(0, (m - 4 * j) * 128)
                            qso = slice(j * QB + off, (j + 1) * QB)
                            sc = psum.tile([128, QB], F32, tag="mm")
                            nc.tensor.matmul(
                                sc[:, off:], kT[h][:, m * 128:(m + 1) * 128],
                                qT[h][:, qso],
                                start=True, stop=False)
                            nc.tensor.matmul(
                                sc[:, off:], ident_bf, slab[:, ml, off:],
                                start=False, stop=True, skip_group_check=True)
                            ex = expp.tile([128, QB], F32R, tag="ex")
                            nc.scalar.activation(
                                ex[:, off:], sc[:, off:],
                                mybir.ActivationFunctionType.Exp)
                            nc.tensor.matmul(
                                pv[:, off:], vv[h][:, m, :], ex[:, off:],
                                start=(m == 0), stop=(m == nk - 1),
                                skip_group_check=True)
                            nc.tensor.matmul(
                                lps[:, off:], t_wr[:, b, m:m + 1], ex[:, off:],
                                start=(m == 0), stop=(m == nk - 1),
                                skip_group_check=True)
                            # interleave pending output-projection units
                            if pending_c and (steps_left <= len(pending_c)
                                              or (m + h) % 2 == 0):
                                emit_c_unit(*pending_c.pop(0))
                            steps_left -= 1
                    rl = normp.tile([1, QB], F32, tag="rl")
                    nc.vector.reciprocal(rl, lps)
                    rb = normp.tile([128, QB], F32, tag="rb")
                    nc.gpsimd.partition_broadcast(rb, rl)
                    nc.vector.tensor_mul(attnT[h][:, qs], pv, rb)
                if j < NQB - 1:
                    for c in pending_c:
                        emit_c_unit(*c)
                    pending_c = []
                pending_c = pending_c + [
                    (attnT, b, st, nb) for st in range(4 * j, 4 * j + 4)
                    for nb in range(D // 512)]
            for c in pending_c:
                emit_c_unit(*c)
            pending_c = []

    nc.compile()
    return nc


def _host_prep(x, Wq, Wk, Wv, Wo, policy_mask, memory_weights):
    """Build the per-core input maps."""
    bf = ml_dtypes.bfloat16
    xhi = x.astype(bf)
    xlo = (x.astype(np.float32) - xhi.astype(np.float32)).astype(bf)

    # RoPE tables, transposed: cos2 = [cosT; cosT], sinpm = [-sinT; sinT]
    inv_freq = (1.0 / (ROPE_BASE ** (np.arange(0, HD, 2, dtype=np.float32) / HD)))
    t = np.arange(S, dtype=np.float32)
    freqs = np.outer(t, inv_freq).astype(np.float32)      # [S, 64]
    cosT = np.cos(freqs).T.astype(np.float32)             # [64, S]
    sinT = np.sin(freqs).T.astype(np.float32)
    cs = np.ascontiguousarray(np.concatenate([sinT, cosT], axis=0))

    # memory multiplier w = 1 + GS*mw + 1e-8  (exp(log1p(z)) = 1+z)
    mw = memory_weights.reshape(B, S).astype(np.float64)
    w = (1.0 + GS * mw + 1e-8).astype(np.float32)

    # transposed, causal-masked, pre-scaled policy bias per head (bf16)
    maskT = np.tril(np.full((S, S), MASK_NEG, dtype=np.float32), -1)
    pol = np.asarray(policy_mask, dtype=np.float32)[0]    # [H, S, S]

    in_maps = []
    for c in range(NCORES):
        cols = slice(c * HPC * HD, (c + 1) * HPC * HD)
        bias_c = np.empty((HPC, S, S), dtype=bf)
        for hl in range(HPC):
            hg = c * HPC + hl
            bias_c[hl] = (GS * pol[hg].T + maskT).astype(bf)
        in_maps.append({
            "xhi": xhi, "xlo": xlo,
            "wq": np.ascontiguousarray(Wq[:, cols]),
            "wk": np.ascontiguousarray(Wk[:, cols] * np.float32(SCALE)),
            "wv": np.ascontiguousarray(Wv[:, cols]),
            "wo": np.ascontiguousarray(Wo[cols, :]),
            "biasT": bias_c,
            "wr": w, "w32": w,
            "cs": cs,
        })
    return in_maps


def kernel(x, Wq, Wk, Wv, Wo, bo, policy_mask, memory_weights):
    x = np.asarray(x, dtype=np.float32)
    Wq = np.asarray(Wq, dtype=np.float32)
    Wk = np.asarray(Wk, dtype=np.float32)
    Wv = np.asarray(Wv, dtype=np.float32)
    Wo = np.asarray(Wo, dtype=np.float32)
    bo = np.asarray(bo, dtype=np.float32)

    if "nc" not in _CACHE:
        _CACHE["nc"] = build_nc()
    nc = _CACHE["nc"]

    in_maps = _host_prep(x, Wq, Wk, Wv, Wo, policy_mask, memory_weights)
    res = run_bass_kernel_spmd(nc, in_maps, core_ids=list(range(NCORES)))

    acc = np.zeros((B, S, D), dtype=np.float64)
    for c in range(NCORES):
        acc += res.results[c]["y"].astype(np.float64)
    return (acc + bo.astype(np.float64)).astype(np.float32)



# revision 4
# speedup vs baseline: 1.2254x; 1.2254x over previous
"""EnhancedGovernanceAttention Trainium2 kernel (8 NeuronCores, SPMD).

Sharding: core c owns heads {2c, 2c+1} for BOTH batches (policy_mask is
per-head and batch-shared, so each policy slice is loaded once per core
and reused for both batches).  Each core computes its heads' attention
and a row-parallel partial of the Wo projection; the host sums the 8
bf16 partials (fp32 accumulate) and adds bo.

Math notes (vs the jax reference):
 - softmax max-subtraction is dropped: scores ~ N(0,1) + small bias, so
   exp() cannot overflow in fp32; softmax is shift-invariant.
 - log1p memory bias: log(w) with w = 1 + GS*mw + 1e-8 is applied as the
   per-partition (k) bias operand of the exp activation, so
   ex = w * exp(s); the denominator is then a plain ones-row matmul.
 - causal mask: only lower-triangle k-tiles are computed; the intra-tile
   diagonal mask is baked into the (bf16) policy bias as -40.
 - scores are computed TRANSPOSED ([k, q]) so the PV matmul directly
   yields attn^T, which is the lhsT the output projection needs.
 - the whole PE pipeline runs in bf16 (x, W, q/k/v, exp, attn, Wo);
   psum accumulation stays fp32.  Measured end-to-end rel err ~2e-3,
   well inside the 2e-2 gate.
"""

import numpy as np
import ml_dtypes
from contextlib import ExitStack

import concourse.bass as bass
import concourse.tile as tile
from concourse import bacc, mybir
from concourse.bass_utils import run_bass_kernel_spmd
from concourse.masks import make_identity

B, S, D, H, HD = 2, 2048, 2048, 16, 128
GS = 0.1
ROPE_BASE = 10000.0
NCORES = 8
HPC = H // NCORES          # heads per core = 2
SCALE = float(HD) ** -0.5
DT = D // 128              # 16 d-tiles
ST = S // 128              # 16 s-tiles (also k-tiles)
QB = 512                   # q-block width (phase B)
NQB = S // QB              # 4 q-blocks
SB = 256                   # s-block width (phase A panels)
NSB = S // SB              # 8 s-blocks
MASK_NEG = -40.0
SLAB_K = 2                 # k-tiles per bias slab load

F32 = mybir.dt.float32
BF16 = mybir.dt.bfloat16

_CACHE = {}


def build_nc():
    nc = bacc.Bacc("TRN2", target_bir_lowering=False, debug=False,
                   num_devices=NCORES)

    d_x = nc.dram_tensor("xbf", [B, S, D], BF16, kind="ExternalInput").ap()
    d_wq = nc.dram_tensor("wq", [D, HPC * HD], BF16, kind="ExternalInput").ap()
    d_wk = nc.dram_tensor("wk", [D, HPC * HD], BF16, kind="ExternalInput").ap()
    d_wv = nc.dram_tensor("wv", [D, HPC * HD], BF16, kind="ExternalInput").ap()
    d_wo = nc.dram_tensor("wo", [HPC * HD, D], BF16, kind="ExternalInput").ap()
    d_bias = nc.dram_tensor("biasT", [HPC, S, S], BF16, kind="ExternalInput").ap()
    d_logw = nc.dram_tensor("logw", [B, S], F32, kind="ExternalInput").ap()
    d_csc = nc.dram_tensor("csc", [128, S], F32, kind="ExternalInput").ap()
    d_csn = nc.dram_tensor("csn", [128, S], F32, kind="ExternalInput").ap()
    d_y = nc.dram_tensor("y", [B, S, D], BF16, kind="ExternalOutput").ap()

    with tile.TileContext(nc) as tc, ExitStack() as ctx:
        consts = ctx.enter_context(tc.tile_pool(name="consts", bufs=1))
        wpool = ctx.enter_context(tc.tile_pool(name="wpool", bufs=1))
        qkv = ctx.enter_context(tc.tile_pool(name="qkv", bufs=1))
        panels = ctx.enter_context(tc.tile_pool(name="panels", bufs=2))
        rope = ctx.enter_context(tc.tile_pool(name="rope", bufs=2))
        slabs = ctx.enter_context(tc.tile_pool(name="slabs", bufs=3))
        expp = ctx.enter_context(tc.tile_pool(name="expp", bufs=4))
        normp = ctx.enter_context(tc.tile_pool(name="normp", bufs=2))
        outp = ctx.enter_context(tc.tile_pool(name="outp", bufs=4))
        # PSUM budget is 8 banks of [128 x 2KB]:
        #   psA (4): score tiles + phase-C matmul tiles + phase-A qkv tiles
        #   psB (2): pv accumulators (b0, b1)
        #   psL (2): denominator accumulators (b0, b1)
        psA = ctx.enter_context(tc.tile_pool(name="psA", bufs=4, space="PSUM"))
        psB = ctx.enter_context(tc.tile_pool(name="psB", bufs=2, space="PSUM"))
        psL = ctx.enter_context(tc.tile_pool(name="psL", bufs=2, space="PSUM"))

        # ------------- constants (emission order = DMA priority) -------------
        t_w = {}
        t_w["wq"] = wpool.tile([128, DT, HPC * HD], BF16, tag="wq", name="wq")
        nc.gpsimd.dma_start(t_w["wq"], d_wq.rearrange("(t p) c -> p t c", p=128))

        # first panel transpose is emitted inside the b=0,sb=0 iteration below

        t_w["wk"] = wpool.tile([128, DT, HPC * HD], BF16, tag="wk", name="wk")
        nc.gpsimd.dma_start(t_w["wk"], d_wk.rearrange("(t p) c -> p t c", p=128))
        t_w["wv"] = wpool.tile([128, DT, HPC * HD], BF16, tag="wv", name="wv")
        nc.gpsimd.dma_start(t_w["wv"], d_wv.rearrange("(t p) c -> p t c", p=128))

        t_csc = consts.tile([128, S], F32, tag="csc")
        nc.gpsimd.dma_start(t_csc, d_csc)
        t_csn = consts.tile([128, S], F32, tag="csn")
        nc.gpsimd.dma_start(t_csn, d_csn)

        ident_bf = consts.tile([128, 128], BF16, tag="ident_bf")
        make_identity(nc, ident_bf)
        ones_bf = consts.tile([128, 1], BF16, tag="ones_bf")
        nc.gpsimd.memset(ones_bf, 1.0)
        t_logw = consts.tile([128, B, ST], F32, tag="logw")
        nc.gpsimd.dma_start(t_logw, d_logw.rearrange("b (t p) -> p b t", p=128))
        t_wo = consts.tile([128, HPC, D], BF16, tag="wo")
        nc.gpsimd.dma_start(t_wo, d_wo.rearrange("(h p) c -> p h c", p=128))

        def emit_c_unit(attnT_ref, b_ref, st, nb):
            ss = slice(st * 128, (st + 1) * 128)
            ns = slice(nb * 512, (nb + 1) * 512)
            ops = psA.tile([128, 512], F32, tag="mm", name="ops")
            for h in range(HPC):
                nc.tensor.matmul(
                    ops, attnT_ref[b_ref][h][:, ss], t_wo[:, h, ns],
                    start=(h == 0), stop=(h == HPC - 1))
            ob = outp.tile([128, 512], BF16, tag="ob")
            nc.gpsimd.tensor_copy(ob, ops)
            eng = nc.scalar if (st + nb) % 2 == 0 else nc.sync
            eng.dma_start(d_y[b_ref, ss, ns], ob)

        # ================= phase A: QKV + RoPE for both batches ==============
        qT = {}
        kT = {}
        vv = {}
        for b in range(B):
            for h in range(HPC):
                qT[b, h] = qkv.tile([128, S], BF16, tag=f"qT{b}{h}", name=f"qT{b}{h}")
                kT[b, h] = qkv.tile([128, S], BF16, tag=f"kT{b}{h}", name=f"kT{b}{h}")
            vv[b] = qkv.tile([128, ST, HPC * HD], BF16, tag=f"v{b}", name=f"v{b}")

        for b in range(B):
            for sb_i in range(NSB):
                s0 = sb_i * SB
                blk = slice(s0, s0 + SB)
                panel = panels.tile([128, DT, SB], BF16, tag="panel", name="panel")
                nc.sync.dma_start_transpose(panel, d_x[b, blk, :])

                for h in range(HPC):
                    hc = slice(h * HD, (h + 1) * HD)
                    # --- q^T and k^T with fused RoPE ---
                    for name, dest in (("wq", qT[b, h]), ("wk", kT[b, h])):
                        ps = psA.tile([128, SB], F32, tag="mm")
                        for dt in range(DT):
                            nc.tensor.matmul(
                                ps, t_w[name][:, dt, hc], panel[:, dt, :],
                                start=(dt == 0), stop=(dt == DT - 1))
                        # RoPE: dest = ps * [cos;cos] + swap(ps) * [-sin;sin]
                        t1 = rope.tile([128, SB], F32, tag="t1")
                        t2 = rope.tile([128, SB], F32, tag="t2")
                        nc.vector.tensor_mul(t1, ps, t_csc[:, blk])
                        nc.gpsimd.tensor_mul(
                            t2[0:64, :], ps[64:128, :], t_csn[0:64, blk])
                        nc.gpsimd.tensor_mul(
                            t2[64:128, :], ps[0:64, :], t_csn[64:128, blk])
                        nc.vector.tensor_add(dest[:, blk], t1, t2)
                # --- v in natural [s, hd] layout (both heads at once) ---
                for c4 in range(SB // 128):
                    stile = sb_i * (SB // 128) + c4
                    ch = slice(c4 * 128, (c4 + 1) * 128)
                    psv = psA.tile([128, HPC * HD], F32, tag="mm")
                    for dt in range(DT):
                        nc.tensor.matmul(
                            psv, panel[:, dt, ch], t_w["wv"][:, dt, :],
                            start=(dt == 0), stop=(dt == DT - 1))
                    nc.scalar.copy(vv[b][:, stile, :], psv)

        # ========== phases B+C software-pipelined over q-blocks ==========
        # attnT[b][h] is written over qT[b,h] after its last read
        attnT = {b: {h: qT[b, h] for h in range(HPC)} for b in range(B)}
        pending_c = []
        for j in range(NQB):
            qs = slice(j * QB, (j + 1) * QB)
            nk = 4 * (j + 1)          # causal: k-tiles 0..nk-1
            for h in range(HPC):
                steps_left = nk * B
                pv = {b: psB.tile([128, QB], F32, tag="pv", name=f"pv{b}")
                      for b in range(B)}
                lps = {b: psL.tile([1, QB], F32, tag="l", name=f"l{b}")
                       for b in range(B)}
                for g in range((nk + SLAB_K - 1) // SLAB_K):
                    n = min(SLAB_K, nk - g * SLAB_K)
                    slab = slabs.tile([128, SLAB_K, QB], BF16, tag="slab")
                    k0 = g * SLAB_K * 128
                    nc.sync.dma_start(
                        slab[:, :n, :],
                        d_bias[h, k0:k0 + n * 128, qs].rearrange(
                            "(m p) q -> p m q", p=128))
                    for ml in range(n):
                        m = g * SLAB_K + ml
                        # columns q < 128*m are fully causal-masked; skip them
                        off = max(0, (m - 4 * j) * 128)
                        qso = slice(j * QB + off, (j + 1) * QB)
                        for b in range(B):
                            sc = psA.tile([128, QB], F32, tag="mm")
                            nc.tensor.matmul(
                                sc[:, off:], kT[b, h][:, m * 128:(m + 1) * 128],
                                qT[b, h][:, qso],
                                start=True, stop=False)
                            nc.tensor.matmul(
                                sc[:, off:], ident_bf, slab[:, ml, off:],
                                start=False, stop=True, skip_group_check=True)
                            ex = expp.tile([128, QB], BF16, tag="ex")
                            nc.scalar.activation(
                                ex[:, off:], sc[:, off:],
                                mybir.ActivationFunctionType.Exp,
                                bias=t_logw[:, b, m:m + 1])
                            nc.tensor.matmul(
                                pv[b][:, off:], vv[b][:, m, h * HD:(h + 1) * HD],
                                ex[:, off:],
                                start=(m == 0), stop=(m == nk - 1),
                                skip_group_check=True)
                            nc.tensor.matmul(
                                lps[b][:, off:], ones_bf, ex[:, off:],
                                start=(m == 0), stop=(m == nk - 1),
                                skip_group_check=True)
                        # interleave pending output-projection units
                        if pending_c and (steps_left <= len(pending_c)
                                          or (m + h) % 2 == 0):
                            emit_c_unit(*pending_c.pop(0))
                        steps_left -= 1
                for b in range(B):
                    rl = normp.tile([1, QB], F32, tag="rl")
                    nc.vector.reciprocal(rl, lps[b])
                    rb = normp.tile([128, QB], F32, tag="rb")
                    nc.gpsimd.partition_broadcast(rb, rl)
                    nc.vector.tensor_mul(attnT[b][h][:, qs], pv[b], rb)
            if j < NQB - 1:
                for c in pending_c:
                    emit_c_unit(*c)
                pending_c = []
            pending_c = pending_c + [
                (attnT, b, st, nb) for st in range(4 * j, 4 * j + 4)
                for b in range(B) for nb in range(D // 512)]
        for c in pending_c:
            emit_c_unit(*c)

    nc.compile()
    return nc


def _host_prep(x, Wq, Wk, Wv, Wo, policy_mask, memory_weights):
    """Build the per-core input maps."""
    bf = ml_dtypes.bfloat16
    xbf = np.asarray(x, dtype=bf)

    # RoPE tables, transposed: csc = [cosT; cosT], csn = [-sinT; sinT]
    inv_freq = (1.0 / (ROPE_BASE ** (np.arange(0, HD, 2, dtype=np.float32) / HD)))
    t = np.arange(S, dtype=np.float32)
    freqs = np.outer(t, inv_freq).astype(np.float32)      # [S, 64]
    cosT = np.cos(freqs).T.astype(np.float32)             # [64, S]
    sinT = np.sin(freqs).T.astype(np.float32)
    csc = np.ascontiguousarray(np.concatenate([cosT, cosT], axis=0))
    csn = np.ascontiguousarray(np.concatenate([-sinT, sinT], axis=0))

    # memory multiplier w = 1 + GS*mw + 1e-8  (exp(log1p(z)) = 1+z)
    mw = memory_weights.reshape(B, S).astype(np.float64)
    logw = np.log(1.0 + GS * mw + 1e-8).astype(np.float32)

    # transposed, causal-masked, pre-scaled policy bias per head (bf16)
    maskT = np.tril(np.full((S, S), MASK_NEG, dtype=np.float32), -1)
    pol = np.asarray(policy_mask, dtype=np.float32)[0]    # [H, S, S]

    in_maps = []
    for c in range(NCORES):
        cols = slice(c * HPC * HD, (c + 1) * HPC * HD)
        bias_c = np.empty((HPC, S, S), dtype=bf)
        for hl in range(HPC):
            hg = c * HPC + hl
            bias_c[hl] = (GS * pol[hg].T + maskT).astype(bf)
        in_maps.append({
            "xbf": xbf,
            "wq": np.ascontiguousarray(Wq[:, cols]).astype(bf),
            "wk": np.ascontiguousarray(Wk[:, cols] * np.float32(SCALE)).astype(bf),
            "wv": np.ascontiguousarray(Wv[:, cols]).astype(bf),
            "wo": np.ascontiguousarray(Wo[cols, :]).astype(bf),
            "biasT": bias_c,
            "logw": logw,
            "csc": csc, "csn": csn,
        })
    return in_maps


def kernel(x, Wq, Wk, Wv, Wo, bo, policy_mask, memory_weights):
    x = np.asarray(x, dtype=np.float32)
    Wq = np.asarray(Wq, dtype=np.float32)
    Wk = np.asarray(Wk, dtype=np.float32)
    Wv = np.asarray(Wv, dtype=np.float32)
    Wo = np.asarray(Wo, dtype=np.float32)
    bo = np.asarray(bo, dtype=np.float32)

    if "nc" not in _CACHE:
        _CACHE["nc"] = build_nc()
    nc = _CACHE["nc"]

    in_maps = _host_prep(x, Wq, Wk, Wv, Wo, policy_mask, memory_weights)
    res = run_bass_kernel_spmd(nc, in_maps, core_ids=list(range(NCORES)))

    acc = np.zeros((B, S, D), dtype=np.float64)
    for c in range(NCORES):
        acc += res.results[c]["y"].astype(np.float64)
    return (acc + bo.astype(np.float64)).astype(np.float32)


# revision 6
# speedup vs baseline: 1.2798x; 1.0444x over previous
"""EnhancedGovernanceAttention Trainium2 kernel (8 NeuronCores, SPMD).

Sharding: core c owns heads {2c, 2c+1} for BOTH batches (policy_mask is
per-head and batch-shared, so each policy slice is loaded once per core
and reused for both batches).  Each core computes its heads' attention
and a row-parallel partial of the Wo projection; the host sums the 8
bf16 partials (fp32 accumulate) and adds bo.

Math notes (vs the jax reference):
 - softmax max-subtraction is dropped: scores ~ N(0,1) + small bias, so
   exp() cannot overflow in fp32; softmax is shift-invariant.
 - log1p memory bias: log(w) with w = 1 + GS*mw + 1e-8 is applied as the
   per-partition (k) bias operand of the exp activation, so
   ex = w * exp(s); the denominator is then a plain ones-row matmul.
 - causal mask: only lower-triangle k-tiles are computed; the intra-tile
   diagonal mask is baked into the (bf16) policy bias as -40.
 - scores are computed TRANSPOSED ([k, q]) so the PV matmul directly
   yields attn^T, which is the lhsT the output projection needs.
 - the whole PE pipeline runs in bf16 (x, W, q/k/v, exp, attn, Wo);
   psum accumulation stays fp32.  Measured end-to-end rel err ~2e-3,
   well inside the 2e-2 gate.
"""

import numpy as np
import ml_dtypes
from contextlib import ExitStack

import concourse.bass as bass
import concourse.tile as tile
from concourse import bacc, mybir
from concourse.bass_utils import run_bass_kernel_spmd
from concourse.masks import make_identity

B, S, D, H, HD = 2, 2048, 2048, 16, 128
GS = 0.1
ROPE_BASE = 10000.0
NCORES = 8
HPC = H // NCORES          # heads per core = 2
SCALE = float(HD) ** -0.5
DT = D // 128              # 16 d-tiles
ST = S // 128              # 16 s-tiles (also k-tiles)
QB = 512                   # q-block width (phase B)
NQB = S // QB              # 4 q-blocks
SB = 256                   # s-block width (phase A panels)
NSB = S // SB              # 8 s-blocks
MASK_NEG = -40.0
SLAB_K = 2                 # k-tiles per bias slab load

F32 = mybir.dt.float32
BF16 = mybir.dt.bfloat16

_CACHE = {}


def build_nc():
    nc = bacc.Bacc("TRN2", target_bir_lowering=False, debug=False,
                   num_devices=NCORES)

    d_x = nc.dram_tensor("xbf", [B, S, D], BF16, kind="ExternalInput").ap()
    d_wq = nc.dram_tensor("wq", [D, HPC * HD], BF16, kind="ExternalInput").ap()
    d_wk = nc.dram_tensor("wk", [D, HPC * HD], BF16, kind="ExternalInput").ap()
    d_wv = nc.dram_tensor("wv", [D, HPC * HD], BF16, kind="ExternalInput").ap()
    d_wo = nc.dram_tensor("wo", [HPC * HD, D], BF16, kind="ExternalInput").ap()
    d_bias = nc.dram_tensor("biasT", [HPC, S, S], BF16, kind="ExternalInput").ap()
    d_logw = nc.dram_tensor("logw", [B, S], F32, kind="ExternalInput").ap()
    d_csc = nc.dram_tensor("csc", [128, S], BF16, kind="ExternalInput").ap()
    d_csn = nc.dram_tensor("csn", [128, S], BF16, kind="ExternalInput").ap()
    d_y = nc.dram_tensor("y", [B, S, D], BF16, kind="ExternalOutput").ap()

    with tile.TileContext(nc) as tc, ExitStack() as ctx:
        consts = ctx.enter_context(tc.tile_pool(name="consts", bufs=1))
        wpool = ctx.enter_context(tc.tile_pool(name="wpool", bufs=1))
        qkv = ctx.enter_context(tc.tile_pool(name="qkv", bufs=1))
        panels = ctx.enter_context(tc.tile_pool(name="panels", bufs=3))
        rope = ctx.enter_context(tc.tile_pool(name="rope", bufs=2))
        slabs = ctx.enter_context(tc.tile_pool(name="slabs", bufs=4))
        expp = ctx.enter_context(tc.tile_pool(name="expp", bufs=6))
        normp = ctx.enter_context(tc.tile_pool(name="normp", bufs=2))
        outp = ctx.enter_context(tc.tile_pool(name="outp", bufs=4))
        # PSUM budget is 8 banks of [128 x 2KB]:
        #   psA (4): score tiles + phase-C matmul tiles + phase-A qkv tiles
        #   psB (2): pv accumulators (b0, b1)
        #   psL (2): denominator accumulators (b0, b1)
        psA = ctx.enter_context(tc.tile_pool(name="psA", bufs=4, space="PSUM"))
        psB = ctx.enter_context(tc.tile_pool(name="psB", bufs=2, space="PSUM"))
        psL = ctx.enter_context(tc.tile_pool(name="psL", bufs=2, space="PSUM"))

        # ------------- constants (emission order = DMA priority) -------------
        # weights go on the SP HWDGE queue ahead of the panel transposes so
        # the DMA device serves them first; cheap consts go via Pool SWDGE.
        t_w = {}
        for name, dram in (("wq", d_wq), ("wk", d_wk), ("wv", d_wv)):
            t = wpool.tile([128, DT, HPC * HD], BF16, tag=name, name=name)
            nc.sync.dma_start(t, dram.rearrange("(t p) c -> p t c", p=128))
            t_w[name] = t

        t_csc = consts.tile([128, S], BF16, tag="csc")
        nc.gpsimd.dma_start(t_csc, d_csc)
        t_csn = consts.tile([128, S], BF16, tag="csn")
        nc.gpsimd.dma_start(t_csn, d_csn)

        ident_bf = consts.tile([128, 128], BF16, tag="ident_bf")
        make_identity(nc, ident_bf)
        ones_bf = consts.tile([128, 1], BF16, tag="ones_bf")
        nc.gpsimd.memset(ones_bf, 1.0)
        t_logw = consts.tile([128, B, ST], F32, tag="logw")
        nc.gpsimd.dma_start(t_logw, d_logw.rearrange("b (t p) -> p b t", p=128))
        t_wo = consts.tile([128, HPC, D], BF16, tag="wo")
        nc.gpsimd.dma_start(t_wo, d_wo.rearrange("(h p) c -> p h c", p=128))

        def emit_c_unit(attnT_ref, b_ref, st, nb):
            ss = slice(st * 128, (st + 1) * 128)
            ns = slice(nb * 512, (nb + 1) * 512)
            ops = psA.tile([128, 512], F32, tag="mm", name="ops")
            for h in range(HPC):
                nc.tensor.matmul(
                    ops, attnT_ref[b_ref][h][:, ss], t_wo[:, h, ns],
                    start=(h == 0), stop=(h == HPC - 1))
            ob = outp.tile([128, 512], BF16, tag="ob")
            nc.vector.tensor_copy(ob, ops)
            eng = nc.scalar if (st + nb) % 2 == 0 else nc.sync
            eng.dma_start(d_y[b_ref, ss, ns], ob)

        # ================= phase A: QKV + RoPE for both batches ==============
        qT = {}
        kT = {}
        vv = {}
        for b in range(B):
            for h in range(HPC):
                qT[b, h] = qkv.tile([128, S], BF16, tag=f"qT{b}{h}", name=f"qT{b}{h}")
                kT[b, h] = qkv.tile([128, S], BF16, tag=f"kT{b}{h}", name=f"kT{b}{h}")
            vv[b] = qkv.tile([128, ST, HPC * HD], BF16, tag=f"v{b}", name=f"v{b}")

        for b in range(B):
            for sb_i in range(NSB):
                s0 = sb_i * SB
                blk = slice(s0, s0 + SB)
                panel = panels.tile([128, DT, SB], BF16, tag="panel", name="panel")
                half = DT // 2
                nc.sync.dma_start_transpose(
                    panel[:, :half, :], d_x[b, blk, :half * 128])
                nc.sync.dma_start_transpose(
                    panel[:, half:, :], d_x[b, blk, half * 128:])

                for h in range(HPC):
                    hc = slice(h * HD, (h + 1) * HD)
                    # --- q^T and k^T with fused RoPE ---
                    for name, dest in (("wq", qT[b, h]), ("wk", kT[b, h])):
                        ps = psA.tile([128, SB], F32, tag="mm")
                        for dt in range(DT):
                            nc.tensor.matmul(
                                ps, t_w[name][:, dt, hc], panel[:, dt, :],
                                start=(dt == 0), stop=(dt == DT - 1))
                        # RoPE: dest = ps * [cos;cos] + swap(ps) * [-sin;sin]
                        t1 = rope.tile([128, SB], F32, tag="t1")
                        t2 = rope.tile([128, SB], F32, tag="t2")
                        nc.vector.tensor_mul(t1, ps, t_csc[:, blk])
                        nc.gpsimd.tensor_mul(
                            t2[0:64, :], ps[64:128, :], t_csn[0:64, blk])
                        nc.gpsimd.tensor_mul(
                            t2[64:128, :], ps[0:64, :], t_csn[64:128, blk])
                        nc.vector.tensor_add(dest[:, blk], t1, t2)
                # --- v in natural [s, hd] layout (both heads at once) ---
                for c4 in range(SB // 128):
                    stile = sb_i * (SB // 128) + c4
                    ch = slice(c4 * 128, (c4 + 1) * 128)
                    psv = psA.tile([128, HPC * HD], F32, tag="mm")
                    for dt in range(DT):
                        nc.tensor.matmul(
                            psv, panel[:, dt, ch], t_w["wv"][:, dt, :],
                            start=(dt == 0), stop=(dt == DT - 1))
                    nc.scalar.copy(vv[b][:, stile, :], psv)

        # ========== phases B+C software-pipelined over q-blocks ==========
        # attnT[b][h] is written over qT[b,h] after its last read
        attnT = {b: {h: qT[b, h] for h in range(HPC)} for b in range(B)}
        pending_c = []
        for j in range(NQB):
            qs = slice(j * QB, (j + 1) * QB)
            nk = 4 * (j + 1)          # causal: k-tiles 0..nk-1
            for h in range(HPC):
                steps_left = nk * B
                pv = {b: psB.tile([128, QB], F32, tag="pv", name=f"pv{b}")
                      for b in range(B)}
                lps = {b: psL.tile([1, QB], F32, tag="l", name=f"l{b}")
                       for b in range(B)}
                for g in range((nk + SLAB_K - 1) // SLAB_K):
                    n = min(SLAB_K, nk - g * SLAB_K)
                    slab = slabs.tile([128, SLAB_K, QB], BF16, tag="slab")
                    k0 = g * SLAB_K * 128
                    nc.sync.dma_start(
                        slab[:, :n, :],
                        d_bias[h, k0:k0 + n * 128, qs].rearrange(
                            "(m p) q -> p m q", p=128))
                    for ml in range(n):
                        m = g * SLAB_K + ml
                        # columns q < 128*m are fully causal-masked; skip them
                        off = max(0, (m - 4 * j) * 128)
                        qso = slice(j * QB + off, (j + 1) * QB)
                        for b in range(B):
                            sc = psA.tile([128, QB], F32, tag="mm")
                            nc.tensor.matmul(
                                sc[:, off:], kT[b, h][:, m * 128:(m + 1) * 128],
                                qT[b, h][:, qso],
                                start=True, stop=False)
                            nc.tensor.matmul(
                                sc[:, off:], ident_bf, slab[:, ml, off:],
                                start=False, stop=True, skip_group_check=True)
                            ex = expp.tile([128, QB], BF16, tag="ex")
                            nc.scalar.activation(
                                ex[:, off:], sc[:, off:],
                                mybir.ActivationFunctionType.Exp,
                                bias=t_logw[:, b, m:m + 1])
                            nc.tensor.matmul(
                                pv[b][:, off:], vv[b][:, m, h * HD:(h + 1) * HD],
                                ex[:, off:],
                                start=(m == 0), stop=(m == nk - 1),
                                skip_group_check=True)
                            nc.tensor.matmul(
                                lps[b][:, off:], ones_bf, ex[:, off:],
                                start=(m == 0), stop=(m == nk - 1),
                                skip_group_check=True)
                        # interleave pending output-projection units
                        if pending_c and (steps_left <= len(pending_c)
                                          or (m + h) % 2 == 0):
                            emit_c_unit(*pending_c.pop(0))
                        steps_left -= 1
                for b in range(B):
                    rl = normp.tile([1, QB], F32, tag="rl")
                    nc.vector.reciprocal(rl, lps[b])
                    rb = normp.tile([128, QB], F32, tag="rb")
                    nc.gpsimd.partition_broadcast(rb, rl)
                    nc.vector.tensor_mul(attnT[b][h][:, qs], pv[b], rb)
            if j < NQB - 1:
                for c in pending_c:
                    emit_c_unit(*c)
                pending_c = []
            pending_c = pending_c + [
                (attnT, b, st, nb) for st in range(4 * j, 4 * j + 4)
                for b in range(B) for nb in range(D // 512)]
        for c in pending_c:
            emit_c_unit(*c)

    nc.compile()
    return nc


def _host_prep(x, Wq, Wk, Wv, Wo, policy_mask, memory_weights):
    """Build the per-core input maps."""
    bf = ml_dtypes.bfloat16
    xbf = np.asarray(x, dtype=bf)

    # RoPE tables, transposed: csc = [cosT; cosT], csn = [-sinT; sinT]
    inv_freq = (1.0 / (ROPE_BASE ** (np.arange(0, HD, 2, dtype=np.float32) / HD)))
    t = np.arange(S, dtype=np.float32)
    freqs = np.outer(t, inv_freq).astype(np.float32)      # [S, 64]
    cosT = np.cos(freqs).T.astype(np.float32)             # [64, S]
    sinT = np.sin(freqs).T.astype(np.float32)
    csc = np.ascontiguousarray(np.concatenate([cosT, cosT], axis=0)).astype(bf)
    csn = np.ascontiguousarray(np.concatenate([-sinT, sinT], axis=0)).astype(bf)

    # memory multiplier w = 1 + GS*mw + 1e-8  (exp(log1p(z)) = 1+z)
    mw = memory_weights.reshape(B, S).astype(np.float64)
    logw = np.log(1.0 + GS * mw + 1e-8).astype(np.float32)

    # transposed, causal-masked, pre-scaled policy bias per head (bf16)
    maskT = np.tril(np.full((S, S), MASK_NEG, dtype=np.float32), -1)
    pol = np.asarray(policy_mask, dtype=np.float32)[0]    # [H, S, S]

    in_maps = []
    for c in range(NCORES):
        cols = slice(c * HPC * HD, (c + 1) * HPC * HD)
        bias_c = np.empty((HPC, S, S), dtype=bf)
        for hl in range(HPC):
            hg = c * HPC + hl
            bias_c[hl] = (GS * pol[hg].T + maskT).astype(bf)
        in_maps.append({
            "xbf": xbf,
            "wq": np.ascontiguousarray(Wq[:, cols]).astype(bf),
            "wk": np.ascontiguousarray(Wk[:, cols] * np.float32(SCALE)).astype(bf),
            "wv": np.ascontiguousarray(Wv[:, cols]).astype(bf),
            "wo": np.ascontiguousarray(Wo[cols, :]).astype(bf),
            "biasT": bias_c,
            "logw": logw,
            "csc": csc, "csn": csn,
        })
    return in_maps


def kernel(x, Wq, Wk, Wv, Wo, bo, policy_mask, memory_weights):
    x = np.asarray(x, dtype=np.float32)
    Wq = np.asarray(Wq, dtype=np.float32)
    Wk = np.asarray(Wk, dtype=np.float32)
    Wv = np.asarray(Wv, dtype=np.float32)
    Wo = np.asarray(Wo, dtype=np.float32)
    bo = np.asarray(bo, dtype=np.float32)

    if "nc" not in _CACHE:
        _CACHE["nc"] = build_nc()
    nc = _CACHE["nc"]

    in_maps = _host_prep(x, Wq, Wk, Wv, Wo, policy_mask, memory_weights)
    res = run_bass_kernel_spmd(nc, in_maps, core_ids=list(range(NCORES)))

    acc = np.zeros((B, S, D), dtype=np.float64)
    for c in range(NCORES):
        acc += res.results[c]["y"].astype(np.float64)
    return (acc + bo.astype(np.float64)).astype(np.float32)


# revision 9
# speedup vs baseline: 1.3222x; 1.0331x over previous
"""EnhancedGovernanceAttention Trainium2 kernel (8 NeuronCores, SPMD).

Sharding: core c owns heads {2c, 2c+1} for BOTH batches (policy_mask is
per-head and batch-shared, so each policy slice is loaded once per core
and reused for both batches).  Each core computes its heads' attention
and a row-parallel partial of the Wo projection; the host sums the 8
bf16 partials (fp32 accumulate) and adds bo.

Math notes (vs the jax reference):
 - softmax max-subtraction is dropped: scores ~ N(0,1) + small bias, so
   exp() cannot overflow in fp32; softmax is shift-invariant.
 - log1p memory bias: log(w) with w = 1 + GS*mw + 1e-8 is applied as the
   per-partition (k) bias operand of the exp activation, so
   ex = w * exp(s); the denominator is then a plain ones-row matmul.
 - the policy bias (with the causal -40 mask baked into the diagonal
   tiles) is added to the scores on the PE via a bf16 identity matmul
   accumulating into the same psum group as the k.q matmul.
 - scores are computed TRANSPOSED ([k, q]) so the PV matmul directly
   yields attn^T, which is the lhsT the output projection needs.
 - attn^T is normalized out of psum by a DVE multiply with the
   partition-broadcast reciprocal of the denominator.
 - the whole PE pipeline runs in bf16 (x, W, q/k/v, exp, attn, Wo);
   psum accumulation stays fp32.
"""

import numpy as np
import ml_dtypes
from contextlib import ExitStack

import concourse.bass as bass
import concourse.tile as tile
from concourse import bacc, mybir
from concourse.bass_utils import run_bass_kernel_spmd
from concourse.masks import make_identity

B, S, D, H, HD = 2, 2048, 2048, 16, 128
GS = 0.1
ROPE_BASE = 10000.0
NCORES = 8
HPC = H // NCORES          # heads per core = 2
SCALE = float(HD) ** -0.5
DT = D // 128              # 16 d-tiles
ST = S // 128              # 16 s-tiles (also k-tiles)
QB = 512                   # q-block width (phase B)
NQB = S // QB              # 4 q-blocks
SB = 256                   # s-block width (phase A panels)
NSB = S // SB              # 8 s-blocks
MASK_NEG = -40.0
SLAB_K = 2                 # k-tiles per bias slab load

F32 = mybir.dt.float32
BF16 = mybir.dt.bfloat16

_CACHE = {}


def build_nc():
    nc = bacc.Bacc("TRN2", target_bir_lowering=False, debug=False,
                   num_devices=NCORES)

    d_x = nc.dram_tensor("xbf", [B, S, D], BF16, kind="ExternalInput").ap()
    d_wq = nc.dram_tensor("wq", [D, HPC * HD], BF16, kind="ExternalInput").ap()
    d_wk = nc.dram_tensor("wk", [D, HPC * HD], BF16, kind="ExternalInput").ap()
    d_wv = nc.dram_tensor("wv", [D, HPC * HD], BF16, kind="ExternalInput").ap()
    d_wo = nc.dram_tensor("wo", [HPC * HD, D], BF16, kind="ExternalInput").ap()
    d_bias = nc.dram_tensor("biasT", [HPC, S, S], BF16, kind="ExternalInput").ap()
    d_logw = nc.dram_tensor("logw", [B, S], F32, kind="ExternalInput").ap()
    d_csc = nc.dram_tensor("csc", [128, S], BF16, kind="ExternalInput").ap()
    d_csn = nc.dram_tensor("csn", [128, S], BF16, kind="ExternalInput").ap()
    d_y = nc.dram_tensor("y", [B, S, D], BF16, kind="ExternalOutput").ap()

    with tile.TileContext(nc) as tc, ExitStack() as ctx:
        consts = ctx.enter_context(tc.tile_pool(name="consts", bufs=1))
        wpool = ctx.enter_context(tc.tile_pool(name="wpool", bufs=1))
        qkv = ctx.enter_context(tc.tile_pool(name="qkv", bufs=1))
        panels = ctx.enter_context(tc.tile_pool(name="panels", bufs=3))
        rope = ctx.enter_context(tc.tile_pool(name="rope", bufs=2))
        slabs = ctx.enter_context(tc.tile_pool(name="slabs", bufs=4))
        expp = ctx.enter_context(tc.tile_pool(name="expp", bufs=6))
        normp = ctx.enter_context(tc.tile_pool(name="normp", bufs=2))
        outp = ctx.enter_context(tc.tile_pool(name="outp", bufs=4))
        # PSUM budget is 8 banks of [128 x 2KB]:
        #   psA (4): bias+score tiles, phase-C matmul tiles, phase-A qkv tiles
        #   psB (2): pv accumulators (b0, b1)
        #   psL (2): denominator accumulators + transposed-reciprocal tiles
        psA = ctx.enter_context(tc.tile_pool(name="psA", bufs=4, space="PSUM"))
        psB = ctx.enter_context(tc.tile_pool(name="psB", bufs=2, space="PSUM"))
        psL = ctx.enter_context(tc.tile_pool(name="psL", bufs=2, space="PSUM"))

        # ------------- constants (emission order = DMA priority) -------------
        # Everything startup-critical goes on the SP HWDGE queue in the exact
        # order the DMA device should serve it; wv/logw/wo are emitted inside
        # the phase-A loop so panel transposes interleave ahead of them.
        t_w = {}
        for name, dram in (("wq", d_wq), ("wk", d_wk)):
            t = wpool.tile([128, DT, HPC * HD], BF16, tag=name, name=name)
            nc.sync.dma_start(t, dram.rearrange("(t p) c -> p t c", p=128))
            t_w[name] = t
        t_w["wv"] = wpool.tile([128, DT, HPC * HD], BF16, tag="wv", name="wv")
        t_csc = consts.tile([128, S], BF16, tag="csc")
        nc.sync.dma_start(t_csc, d_csc)
        t_csn = consts.tile([128, S], BF16, tag="csn")
        nc.sync.dma_start(t_csn, d_csn)

        ones_bf = consts.tile([128, 1], BF16, tag="ones_bf")
        nc.gpsimd.memset(ones_bf, 1.0)
        ident_bf = consts.tile([128, 128], BF16, tag="ident_bf")
        make_identity(nc, ident_bf)
        t_logw = consts.tile([128, B, ST], F32, tag="logw")
        t_wo = consts.tile([128, HPC, D], BF16, tag="wo")

        def emit_c_unit(b_ref, st, nb):
            ss = slice(st * 128, (st + 1) * 128)
            ns = slice(nb * 512, (nb + 1) * 512)
            ops = psA.tile([128, 512], F32, tag="mm", name="ops")
            for h in range(HPC):
                nc.tensor.matmul(
                    ops, attnT[b_ref, h][:, ss], t_wo[:, h, ns],
                    start=(h == 0), stop=(h == HPC - 1))
            ob = outp.tile([128, 512], BF16, tag="ob")
            nc.vector.tensor_copy(ob, ops)
            eng = nc.scalar if (st + nb) % 2 == 0 else nc.sync
            eng.dma_start(d_y[b_ref, ss, ns], ob)

        # ================= phase A: QKV + RoPE for both batches ==============
        qT = {}
        kT = {}
        vv = {}
        for b in range(B):
            for h in range(HPC):
                qT[b, h] = qkv.tile([128, S], BF16, tag=f"qT{b}{h}", name=f"qT{b}{h}")
                kT[b, h] = qkv.tile([128, S], BF16, tag=f"kT{b}{h}", name=f"kT{b}{h}")
            vv[b] = qkv.tile([128, ST, HPC * HD], BF16, tag=f"v{b}", name=f"v{b}")

        for b in range(B):
            for sb_i in range(NSB):
                s0 = sb_i * SB
                blk = slice(s0, s0 + SB)
                panel = panels.tile([128, DT, SB], BF16, tag="panel", name="panel")
                half = DT // 2
                nc.sync.dma_start_transpose(
                    panel[:, :half, :], d_x[b, blk, :half * 128])
                nc.sync.dma_start_transpose(
                    panel[:, half:, :], d_x[b, blk, half * 128:])
                if b == 0 and sb_i == 0:
                    nc.sync.dma_start(
                        t_w["wv"], d_wv.rearrange("(t p) c -> p t c", p=128))
                if b == 0 and sb_i == 1:
                    nc.sync.dma_start(
                        t_logw, d_logw.rearrange("b (t p) -> p b t", p=128))
                    nc.sync.dma_start(
                        t_wo, d_wo.rearrange("(h p) c -> p h c", p=128))

                for h in range(HPC):
                    hc = slice(h * HD, (h + 1) * HD)
                    # --- q^T and k^T with fused RoPE ---
                    for name, dest in (("wq", qT[b, h]), ("wk", kT[b, h])):
                        ps = psA.tile([128, SB], F32, tag="mm")
                        for dt in range(DT):
                            nc.tensor.matmul(
                                ps, t_w[name][:, dt, hc], panel[:, dt, :],
                                start=(dt == 0), stop=(dt == DT - 1))
                        # RoPE: dest = ps * [cos;cos] + swap(ps) * [-sin;sin]
                        t1 = rope.tile([128, SB], F32, tag="t1")
                        t2 = rope.tile([128, SB], F32, tag="t2")
                        nc.vector.tensor_mul(t1, ps, t_csc[:, blk])
                        nc.gpsimd.tensor_mul(
                            t2[0:64, :], ps[64:128, :], t_csn[0:64, blk])
                        nc.gpsimd.tensor_mul(
                            t2[64:128, :], ps[0:64, :], t_csn[64:128, blk])
                        nc.vector.tensor_add(dest[:, blk], t1, t2)
                # --- v in natural [s, hd] layout (both heads at once) ---
                for c4 in range(SB // 128):
                    stile = sb_i * (SB // 128) + c4
                    ch = slice(c4 * 128, (c4 + 1) * 128)
                    psv = psA.tile([128, HPC * HD], F32, tag="mm")
                    for dt in range(DT):
                        nc.tensor.matmul(
                            psv, panel[:, dt, ch], t_w["wv"][:, dt, :],
                            start=(dt == 0), stop=(dt == DT - 1))
                    nc.scalar.copy(vv[b][:, stile, :], psv)

        # ========== phases B+C software-pipelined over q-blocks ==========
        # attnT (normalized) overwrites qT[b,h][:, qs] after its last read
        attnT = {(b, h): qT[b, h] for b in range(B) for h in range(HPC)}
        pending_c = []
        for j in range(NQB):
            qs = slice(j * QB, (j + 1) * QB)
            nk = 4 * (j + 1)          # causal: k-tiles 0..nk-1
            for h in range(HPC):
                steps_left = nk * B
                pv = {b: psB.tile([128, QB], F32, tag="pv", name=f"pv{b}")
                      for b in range(B)}
                lps = {b: psL.tile([1, QB], F32, tag="l", name=f"l{b}")
                       for b in range(B)}
                for g in range((nk + SLAB_K - 1) // SLAB_K):
                  n = min(SLAB_K, nk - g * SLAB_K)
                  slab = slabs.tile([128, SLAB_K, QB], BF16, tag="slab")
                  k0 = g * SLAB_K * 128
                  nc.sync.dma_start(
                      slab[:, :n, :],
                      d_bias[h, k0:k0 + n * 128, qs].rearrange(
                          "(m p) q -> p m q", p=128))
                  for ml in range(n):
                    m = g * SLAB_K + ml
                    # columns q < 128*m are fully causal-masked; skip them
                    off = max(0, (m - 4 * j) * 128)
                    qso = slice(j * QB + off, (j + 1) * QB)
                    for b in range(B):
                        sc = psA.tile([128, QB], F32, tag="mm")
                        nc.tensor.matmul(
                            sc[:, off:], kT[b, h][:, m * 128:(m + 1) * 128],
                            qT[b, h][:, qso],
                            start=True, stop=False)
                        nc.tensor.matmul(
                            sc[:, off:], ident_bf, slab[:, ml, off:],
                            start=False, stop=True, skip_group_check=True)
                        ex = expp.tile([128, QB], BF16, tag="ex")
                        nc.scalar.activation(
                            ex[:, off:], sc[:, off:],
                            mybir.ActivationFunctionType.Exp,
                            bias=t_logw[:, b, m:m + 1])
                        nc.tensor.matmul(
                            pv[b][:, off:], vv[b][:, m, h * HD:(h + 1) * HD],
                            ex[:, off:],
                            start=(m == 0), stop=(m == nk - 1),
                            skip_group_check=True)
                        nc.tensor.matmul(
                            lps[b][:, off:], ones_bf, ex[:, off:],
                            start=(m == 0), stop=(m == nk - 1),
                            skip_group_check=True)
                    # interleave pending output-projection units
                    npop = 2 if len(pending_c) > 16 else 1
                    for _ in range(npop):
                        if pending_c and (steps_left <= len(pending_c)
                                          or (m + h) % 2 == 0 or npop > 1):
                            emit_c_unit(*pending_c.pop(0))
                    steps_left -= 1
                for b in range(B):
                    rl = normp.tile([1, QB], F32, tag="rl")
                    nc.vector.reciprocal(rl, lps[b])
                    rb = normp.tile([128, QB], F32, tag="rb")
                    nc.gpsimd.partition_broadcast(rb, rl)
                    nc.vector.tensor_mul(attnT[b, h][:, qs], pv[b], rb)
            if j < NQB - 1:
                for c in pending_c:
                    emit_c_unit(*c)
                pending_c = []
            pending_c = pending_c + [
                (b, st, nb) for st in range(4 * j, 4 * j + 4)
                for b in range(B) for nb in range(D // 512)]
        for c in pending_c:
            emit_c_unit(*c)

    nc.compile()
    return nc


def _host_prep(x, Wq, Wk, Wv, Wo, policy_mask, memory_weights):
    """Build the per-core input maps."""
    bf = ml_dtypes.bfloat16
    xbf = np.asarray(x, dtype=bf)

    # RoPE tables, transposed: csc = [cosT; cosT], csn = [-sinT; sinT]
    inv_freq = (1.0 / (ROPE_BASE ** (np.arange(0, HD, 2, dtype=np.float32) / HD)))
    t = np.arange(S, dtype=np.float32)
    freqs = np.outer(t, inv_freq).astype(np.float32)      # [S, 64]
    cosT = np.cos(freqs).T.astype(np.float32)             # [64, S]
    sinT = np.sin(freqs).T.astype(np.float32)
    csc = np.ascontiguousarray(np.concatenate([cosT, cosT], axis=0)).astype(bf)
    csn = np.ascontiguousarray(np.concatenate([-sinT, sinT], axis=0)).astype(bf)

    # memory multiplier w = 1 + GS*mw + 1e-8  (exp(log1p(z)) = 1+z)
    mw = memory_weights.reshape(B, S).astype(np.float64)
    logw = np.log(1.0 + GS * mw + 1e-8).astype(np.float32)

    # transposed, causal-masked, pre-scaled policy bias per head (bf16)
    maskT = np.tril(np.full((S, S), MASK_NEG, dtype=np.float32), -1)
    pol = np.asarray(policy_mask, dtype=np.float32)[0]    # [H, S, S]

    in_maps = []
    for c in range(NCORES):
        cols = slice(c * HPC * HD, (c + 1) * HPC * HD)
        bias_c = np.empty((HPC, S, S), dtype=bf)
        for hl in range(HPC):
            hg = c * HPC + hl
            bias_c[hl] = (GS * pol[hg].T + maskT).astype(bf)
        in_maps.append({
            "xbf": xbf,
            "wq": np.ascontiguousarray(Wq[:, cols]).astype(bf),
            "wk": np.ascontiguousarray(Wk[:, cols] * np.float32(SCALE)).astype(bf),
            "wv": np.ascontiguousarray(Wv[:, cols]).astype(bf),
            "wo": np.ascontiguousarray(Wo[cols, :]).astype(bf),
            "biasT": bias_c,
            "logw": logw,
            "csc": csc, "csn": csn,
        })
    return in_maps


def kernel(x, Wq, Wk, Wv, Wo, bo, policy_mask, memory_weights):
    x = np.asarray(x, dtype=np.float32)
    Wq = np.asarray(Wq, dtype=np.float32)
    Wk = np.asarray(Wk, dtype=np.float32)
    Wv = np.asarray(Wv, dtype=np.float32)
    Wo = np.asarray(Wo, dtype=np.float32)
    bo = np.asarray(bo, dtype=np.float32)

    if "nc" not in _CACHE:
        _CACHE["nc"] = build_nc()
    nc = _CACHE["nc"]

    in_maps = _host_prep(x, Wq, Wk, Wv, Wo, policy_mask, memory_weights)
    res = run_bass_kernel_spmd(nc, in_maps, core_ids=list(range(NCORES)))

    acc = np.zeros((B, S, D), dtype=np.float64)
    for c in range(NCORES):
        acc += res.results[c]["y"].astype(np.float64)
    return (acc + bo.astype(np.float64)).astype(np.float32)


# revision 11
# speedup vs baseline: 1.3486x; 1.0200x over previous
"""EnhancedGovernanceAttention Trainium2 kernel (8 NeuronCores, SPMD).

Sharding: core c owns heads {2c, 2c+1} for BOTH batches (policy_mask is
per-head and batch-shared, so each policy slice is loaded once per core
and reused for both batches).  Each core computes its heads' attention
and a row-parallel partial of the Wo projection; the host sums the 8
bf16 partials (fp32 accumulate) and adds bo.

Math notes (vs the jax reference):
 - softmax max-subtraction is dropped: scores ~ N(0,1) + small bias, so
   exp() cannot overflow in fp32; softmax is shift-invariant.
 - log1p memory bias: log(w) with w = 1 + GS*mw + 1e-8 is applied as the
   per-partition (k) bias operand of the exp activation, so
   ex = w * exp(s); the denominator is then a plain ones-row matmul.
 - the policy bias (with the causal -40 mask baked into the diagonal
   tiles) is added to the scores on the PE via a bf16 identity matmul
   accumulating into the same psum group as the k.q matmul.
 - scores are computed TRANSPOSED ([k, q]) so the PV matmul directly
   yields attn^T, which is the lhsT the output projection needs.
 - attn^T is normalized out of psum by a DVE multiply with the
   partition-broadcast reciprocal of the denominator.
 - the whole PE pipeline runs in bf16 (x, W, q/k/v, exp, attn, Wo);
   psum accumulation stays fp32.
"""

import numpy as np
import ml_dtypes
from contextlib import ExitStack

import concourse.bass as bass
import concourse.tile as tile
from concourse import bacc, mybir
from concourse.bass_utils import run_bass_kernel_spmd
from concourse.masks import make_identity

B, S, D, H, HD = 2, 2048, 2048, 16, 128
GS = 0.1
ROPE_BASE = 10000.0
NCORES = 8
HPC = H // NCORES          # heads per core = 2
SCALE = float(HD) ** -0.5
DT = D // 128              # 16 d-tiles
ST = S // 128              # 16 s-tiles (also k-tiles)
QB = 512                   # q-block width (phase B)
NQB = S // QB              # 4 q-blocks
SB = 256                   # s-block width (phase A panels)
NSB = S // SB              # 8 s-blocks
MASK_NEG = -40.0
SLAB_K = 2                 # k-tiles per bias slab load

F32 = mybir.dt.float32
BF16 = mybir.dt.bfloat16

_CACHE = {}


def build_nc():
    nc = bacc.Bacc("TRN2", target_bir_lowering=False, debug=False,
                   num_devices=NCORES)

    d_x = nc.dram_tensor("xbf", [B, S, D], BF16, kind="ExternalInput").ap()
    d_wq = nc.dram_tensor("wq", [D, HPC * HD], BF16, kind="ExternalInput").ap()
    d_wk = nc.dram_tensor("wk", [D, HPC * HD], BF16, kind="ExternalInput").ap()
    d_wv = nc.dram_tensor("wv", [D, HPC * HD], BF16, kind="ExternalInput").ap()
    d_wo = nc.dram_tensor("wo", [HPC * HD, D], BF16, kind="ExternalInput").ap()
    d_bias = nc.dram_tensor("biasT", [HPC, S, S], BF16, kind="ExternalInput").ap()
    d_logw = nc.dram_tensor("logw", [B, S], F32, kind="ExternalInput").ap()
    d_csc = nc.dram_tensor("csc", [128, S], BF16, kind="ExternalInput").ap()
    d_csn = nc.dram_tensor("csn", [128, S], BF16, kind="ExternalInput").ap()
    d_y = nc.dram_tensor("y", [B, S, D], BF16, kind="ExternalOutput").ap()

    with tile.TileContext(nc) as tc, ExitStack() as ctx:
        consts = ctx.enter_context(tc.tile_pool(name="consts", bufs=1))
        wpool = ctx.enter_context(tc.tile_pool(name="wpool", bufs=1))
        qkv = ctx.enter_context(tc.tile_pool(name="qkv", bufs=1))
        panels = ctx.enter_context(tc.tile_pool(name="panels", bufs=3))
        rope = ctx.enter_context(tc.tile_pool(name="rope", bufs=2))
        slabs = ctx.enter_context(tc.tile_pool(name="slabs", bufs=4))
        expp = ctx.enter_context(tc.tile_pool(name="expp", bufs=6))
        normp = ctx.enter_context(tc.tile_pool(name="normp", bufs=2))
        outp = ctx.enter_context(tc.tile_pool(name="outp", bufs=4))
        # PSUM budget is 8 banks of [128 x 2KB]:
        #   psA (4): bias+score tiles, phase-C matmul tiles, phase-A qkv tiles
        #   psB (2): pv accumulators (b0, b1)
        #   psL (2): denominator accumulators + transposed-reciprocal tiles
        psA = ctx.enter_context(tc.tile_pool(name="psA", bufs=4, space="PSUM"))
        psB = ctx.enter_context(tc.tile_pool(name="psB", bufs=2, space="PSUM"))
        psL = ctx.enter_context(tc.tile_pool(name="psL", bufs=2, space="PSUM"))

        # ------------- constants (emission order = DMA priority) -------------
        # Everything startup-critical goes on the SP HWDGE queue in the exact
        # order the DMA device should serve it; wv/logw/wo are emitted inside
        # the phase-A loop so panel transposes interleave ahead of them.
        t_w = {}
        for name, dram in (("wq", d_wq), ("wk", d_wk)):
            t = wpool.tile([128, DT, HPC * HD], BF16, tag=name, name=name)
            nc.sync.dma_start(t, dram.rearrange("(t p) c -> p t c", p=128))
            t_w[name] = t
        t_w["wv"] = wpool.tile([128, DT, HPC * HD], BF16, tag="wv", name="wv")
        t_csc = consts.tile([128, S], BF16, tag="csc")
        nc.sync.dma_start(t_csc, d_csc)
        t_csn = consts.tile([128, S], BF16, tag="csn")
        nc.sync.dma_start(t_csn, d_csn)

        ones_bf = consts.tile([128, 1], BF16, tag="ones_bf")
        nc.gpsimd.memset(ones_bf, 1.0)
        ident_bf = consts.tile([128, 128], BF16, tag="ident_bf")
        make_identity(nc, ident_bf)
        t_logw = consts.tile([128, B, ST], F32, tag="logw")
        t_wo = consts.tile([128, HPC, D], BF16, tag="wo")

        # y writes are issued one C-unit late so the DMA's wait on the ob
        # copy is already satisfied when it reaches the issuing sequencer
        # (an unsatisfied wait would hold the SEQ and block later issues).
        y_lag = []

        def flush_y(nmax=1):
            for _ in range(min(nmax, len(y_lag))):
                eng, dst, ob = y_lag.pop(0)
                eng.dma_start(dst, ob)

        def emit_c_unit(b_ref, st, nb):
            ss = slice(st * 128, (st + 1) * 128)
            ns = slice(nb * 512, (nb + 1) * 512)
            ops = psA.tile([128, 512], F32, tag="mm", name="ops")
            for h in range(HPC):
                nc.tensor.matmul(
                    ops, attnT[b_ref, h][:, ss], t_wo[:, h, ns],
                    start=(h == 0), stop=(h == HPC - 1))
            ob = outp.tile([128, 512], BF16, tag="ob")
            ceng = nc.vector if (st + nb) % 2 == 0 else nc.scalar
            ceng.tensor_copy(ob, ops) if ceng is nc.vector else ceng.copy(ob, ops)
            eng = nc.scalar if (st + nb) % 2 == 0 else nc.sync
            y_lag.append((eng, d_y[b_ref, ss, ns], ob))
            flush_y(1) if len(y_lag) > 2 else None

        # ================= phase A: QKV + RoPE for both batches ==============
        qT = {}
        kT = {}
        vv = {}
        for b in range(B):
            for h in range(HPC):
                qT[b, h] = qkv.tile([128, S], BF16, tag=f"qT{b}{h}", name=f"qT{b}{h}")
                kT[b, h] = qkv.tile([128, S], BF16, tag=f"kT{b}{h}", name=f"kT{b}{h}")
            vv[b] = qkv.tile([128, ST, HPC * HD], BF16, tag=f"v{b}", name=f"v{b}")

        for b in range(B):
            for sb_i in range(NSB):
                s0 = sb_i * SB
                blk = slice(s0, s0 + SB)
                panel = panels.tile([128, DT, SB], BF16, tag="panel", name="panel")
                half = DT // 2
                nc.sync.dma_start_transpose(
                    panel[:, :half, :], d_x[b, blk, :half * 128])
                nc.sync.dma_start_transpose(
                    panel[:, half:, :], d_x[b, blk, half * 128:])
                if b == 0 and sb_i == 0:
                    nc.sync.dma_start(
                        t_w["wv"], d_wv.rearrange("(t p) c -> p t c", p=128))
                if b == 0 and sb_i == 1:
                    nc.sync.dma_start(
                        t_logw, d_logw.rearrange("b (t p) -> p b t", p=128))
                    nc.sync.dma_start(
                        t_wo, d_wo.rearrange("(h p) c -> p h c", p=128))

                for h in range(HPC):
                    hc = slice(h * HD, (h + 1) * HD)
                    # --- q^T and k^T with fused RoPE ---
                    for name, dest in (("wq", qT[b, h]), ("wk", kT[b, h])):
                        ps = psA.tile([128, SB], F32, tag="mm")
                        for dt in range(DT):
                            nc.tensor.matmul(
                                ps, t_w[name][:, dt, hc], panel[:, dt, :],
                                start=(dt == 0), stop=(dt == DT - 1))
                        # RoPE: dest = ps * [cos;cos] + swap(ps) * [-sin;sin]
                        t1 = rope.tile([128, SB], F32, tag="t1")
                        t2 = rope.tile([128, SB], F32, tag="t2")
                        nc.vector.tensor_mul(t1, ps, t_csc[:, blk])
                        nc.vector.tensor_mul(
                            t2[0:64, :], ps[64:128, :], t_csn[0:64, blk])
                        nc.vector.tensor_mul(
                            t2[64:128, :], ps[0:64, :], t_csn[64:128, blk])
                        nc.gpsimd.tensor_add(dest[:, blk], t1, t2)
                # --- v in natural [s, hd] layout (both heads at once) ---
                for c4 in range(SB // 128):
                    stile = sb_i * (SB // 128) + c4
                    ch = slice(c4 * 128, (c4 + 1) * 128)
                    psv = psA.tile([128, HPC * HD], F32, tag="mm")
                    for dt in range(DT):
                        nc.tensor.matmul(
                            psv, panel[:, dt, ch], t_w["wv"][:, dt, :],
                            start=(dt == 0), stop=(dt == DT - 1))
                    nc.scalar.copy(vv[b][:, stile, :], psv)

        # ========== phases B+C software-pipelined over q-blocks ==========
        # attnT (normalized) overwrites qT[b,h][:, qs] after its last read
        attnT = {(b, h): qT[b, h] for b in range(B) for h in range(HPC)}
        pending_c = []
        for j in range(NQB):
            qs = slice(j * QB, (j + 1) * QB)
            nk = 4 * (j + 1)          # causal: k-tiles 0..nk-1
            for h in range(HPC):
                steps_left = nk * B
                pv = {b: psB.tile([128, QB], F32, tag="pv", name=f"pv{b}")
                      for b in range(B)}
                lps = {b: psL.tile([1, QB], F32, tag="l", name=f"l{b}")
                       for b in range(B)}
                for g in range((nk + SLAB_K - 1) // SLAB_K):
                  n = min(SLAB_K, nk - g * SLAB_K)
                  slab = slabs.tile([128, SLAB_K, QB], BF16, tag="slab")
                  k0 = g * SLAB_K * 128
                  nc.sync.dma_start(
                      slab[:, :n, :],
                      d_bias[h, k0:k0 + n * 128, qs].rearrange(
                          "(m p) q -> p m q", p=128))
                  for ml in range(n):
                    m = g * SLAB_K + ml
                    # columns q < 128*m are fully causal-masked; skip them
                    off = max(0, (m - 4 * j) * 128)
                    qso = slice(j * QB + off, (j + 1) * QB)
                    exs = {}
                    for b in range(B):
                        sc = psA.tile([128, QB], F32, tag="mm")
                        nc.tensor.matmul(
                            sc[:, off:], kT[b, h][:, m * 128:(m + 1) * 128],
                            qT[b, h][:, qso],
                            start=True, stop=False)
                        nc.tensor.matmul(
                            sc[:, off:], ident_bf, slab[:, ml, off:],
                            start=False, stop=True, skip_group_check=True)
                        ex = expp.tile([128, QB], BF16, tag="ex")
                        nc.scalar.activation(
                            ex[:, off:], sc[:, off:],
                            mybir.ActivationFunctionType.Exp,
                            bias=t_logw[:, b, m:m + 1])
                        exs[b] = ex
                    for b in range(B):
                        nc.tensor.matmul(
                            pv[b][:, off:], vv[b][:, m, h * HD:(h + 1) * HD],
                            exs[b][:, off:],
                            start=(m == 0), stop=(m == nk - 1),
                            skip_group_check=True)
                        nc.tensor.matmul(
                            lps[b][:, off:], ones_bf, exs[b][:, off:],
                            start=(m == 0), stop=(m == nk - 1),
                            skip_group_check=True)
                    # interleave pending output-projection units
                    npop = 2 if len(pending_c) > 16 else 1
                    for _ in range(npop):
                        if pending_c and (steps_left <= len(pending_c)
                                          or (m + h) % 2 == 0 or npop > 1):
                            emit_c_unit(*pending_c.pop(0))
                    steps_left -= 1
                for b in range(B):
                    rl = normp.tile([1, QB], F32, tag="rl")
                    nc.vector.reciprocal(rl, lps[b])
                    rb = normp.tile([128, QB], F32, tag="rb")
                    nc.gpsimd.partition_broadcast(rb, rl)
                    nc.vector.tensor_mul(attnT[b, h][:, qs], pv[b], rb)
            pending_c = pending_c + [
                (b, st, nb) for st in range(4 * j, 4 * j + 4)
                for b in range(B) for nb in range(D // 512)]
        for c in pending_c:
            emit_c_unit(*c)
        flush_y(len(y_lag))

    nc.compile()
    return nc


def _host_prep(x, Wq, Wk, Wv, Wo, policy_mask, memory_weights):
    """Build the per-core input maps."""
    bf = ml_dtypes.bfloat16
    xbf = np.asarray(x, dtype=bf)

    # RoPE tables, transposed: csc = [cosT; cosT], csn = [-sinT; sinT]
    inv_freq = (1.0 / (ROPE_BASE ** (np.arange(0, HD, 2, dtype=np.float32) / HD)))
    t = np.arange(S, dtype=np.float32)
    freqs = np.outer(t, inv_freq).astype(np.float32)      # [S, 64]
    cosT = np.cos(freqs).T.astype(np.float32)             # [64, S]
    sinT = np.sin(freqs).T.astype(np.float32)
    csc = np.ascontiguousarray(np.concatenate([cosT, cosT], axis=0)).astype(bf)
    csn = np.ascontiguousarray(np.concatenate([-sinT, sinT], axis=0)).astype(bf)

    # memory multiplier w = 1 + GS*mw + 1e-8  (exp(log1p(z)) = 1+z)
    mw = memory_weights.reshape(B, S).astype(np.float64)
    logw = np.log(1.0 + GS * mw + 1e-8).astype(np.float32)

    # transposed, causal-masked, pre-scaled policy bias per head (bf16)
    maskT = np.tril(np.full((S, S), MASK_NEG, dtype=np.float32), -1)
    pol = np.asarray(policy_mask, dtype=np.float32)[0]    # [H, S, S]

    in_maps = []
    for c in range(NCORES):
        cols = slice(c * HPC * HD, (c + 1) * HPC * HD)
        bias_c = np.empty((HPC, S, S), dtype=bf)
        for hl in range(HPC):
            hg = c * HPC + hl
            bias_c[hl] = (GS * pol[hg].T + maskT).astype(bf)
        in_maps.append({
            "xbf": xbf,
            "wq": np.ascontiguousarray(Wq[:, cols]).astype(bf),
            "wk": np.ascontiguousarray(Wk[:, cols] * np.float32(SCALE)).astype(bf),
            "wv": np.ascontiguousarray(Wv[:, cols]).astype(bf),
            "wo": np.ascontiguousarray(Wo[cols, :]).astype(bf),
            "biasT": bias_c,
            "logw": logw,
            "csc": csc, "csn": csn,
        })
    return in_maps


def kernel(x, Wq, Wk, Wv, Wo, bo, policy_mask, memory_weights):
    x = np.asarray(x, dtype=np.float32)
    Wq = np.asarray(Wq, dtype=np.float32)
    Wk = np.asarray(Wk, dtype=np.float32)
    Wv = np.asarray(Wv, dtype=np.float32)
    Wo = np.asarray(Wo, dtype=np.float32)
    bo = np.asarray(bo, dtype=np.float32)

    if "nc" not in _CACHE:
        _CACHE["nc"] = build_nc()
    nc = _CACHE["nc"]

    in_maps = _host_prep(x, Wq, Wk, Wv, Wo, policy_mask, memory_weights)
    res = run_bass_kernel_spmd(nc, in_maps, core_ids=list(range(NCORES)))

    acc = np.zeros((B, S, D), dtype=np.float64)
    for c in range(NCORES):
        acc += res.results[c]["y"].astype(np.float64)
    return (acc + bo.astype(np.float64)).astype(np.float32)
